# revision 19
# baseline (speedup 1.0000x reference)
"""Trainium2 Bass kernel for AdaptiveContourConv (B=4, 128->256ch, 64x64).

Sharding: 8 cores = batch(4) x H-half(2); each core computes output rows
[r0, r0+32) of one sample. All halos come from host-side sharding -> no
inter-core communication.

Per-core pipeline (matmuls in bf16 on the PE):
  conv1 (oc1|mc1 stacked)  -> h|hm            [PE + ACT(BN+ReLU)]
  conv2 (oc2|mc2 stacked)  -> offsets, mask   [PE + ACT]
  PE-transpose offs/mask to hw-major; bilinear corner idx + weights [DVE]
  per-kk: ONE dma_gather (4864 idx: top|bottom corner-pair rows) from
          the padded pixel-major x slab; 2 wide in-place DVE muls apply
          premultiplied corner weights (mask folded); 4 matmul-accum
          "transposes" (rhs=identity) per 128-sample group sum the 4
          corners in PSUM and yield channel-major val    [Pool+DVE+PE]
  einsum with dc_w (9 accumulated K=128 matmuls)                  [PE]
  contour: depthwise via diagonal matmuls + 1x1                   [PE]
  attention: M=49 (dx,dy) matmuls over 3 channel blocks; dx-fold on
     DVE (contiguous partition blocks); dy-fold via K=1 accumulated
     matmuls on contiguous shifted windows; sigmoid; replicate; attn
     multiply commuted past the fusion 1x1                   [PE+DVE]
  fusion matmul + BN + ReLU -> out                            [PE+ACT]
"""

import numpy as np

# ---------------- problem constants ----------------
B, C_IN, C_OUT, H, W, K = 4, 128, 256, 64, 64, 3
KK = K * K
MID = C_IN // 4
EPS = 1e-5

# ---------------- per-core geometry ----------------
R_OUT = 32          # output rows per core
R_CMB = 38          # main_feat/contour rows  [r0-3, r1+3)
R_HM = 40           # h/hm rows               [r0-4, r1+4)
R_X = 42            # c-major x slab rows     [r0-5, r1+5)
XC_W = 66           # padded width (+-1) for conv input
CMB_W = 70          # padded width (+-3) for combined (attn 7x7)
SLAB_R = 48         # pixel-major slab rows   [r0-8, r1+8)
SLAB_W = 72         # pixel-major slab cols   [-4, 68)
HW_CMB = R_CMB * W            # 2432
NS = KK * HW_CMB              # samples = 21888
NGK = HW_CMB // 128           # groups per kk = 19
N_SLAB = SLAB_R * SLAB_W      # 3456 slab pixels
NI2 = 2 * HW_CMB              # idxs per gather (t|b) = 4864
ACMB = R_OUT * CMB_W          # attn/fusion flat window = 2240


def _f32(x):
    return np.ascontiguousarray(np.asarray(x), dtype=np.float32)


def _bf16(x):
    import ml_dtypes
    return np.ascontiguousarray(
        np.asarray(x, dtype=np.float32).astype(ml_dtypes.bfloat16))


def build_bass(stage=99, loop_n=1, debug_out=False):
    import contextlib
    import concourse.bass as bass
    import concourse.mybir as mybir
    import concourse.tile as tile
    from concourse import bacc
    from concourse.ap import AP
    from concourse.masks import make_identity

    dt = mybir.dt
    Alu = mybir.AluOpType
    Act = mybir.ActivationFunctionType

    nc = bacc.Bacc("TRN2", target_bir_lowering=False, debug=False)

    # ---------------- DRAM parameters ----------------
    def P(name, shape, dtype=dt.bfloat16):
        return nc.declare_dram_parameter(name, shape, dtype, isOutput=False)

    xc_d = P("xc", [C_IN, 1 + R_X * XC_W + 1])        # c-major padded x slab
    xs_d = P("xs", [N_SLAB + 1, C_IN])                # pixel-major padded slab
    w1_d = P("w1", [C_IN, KK, 64])                    # conv1 lhsT [c][tap][m]
    s1_d = P("s1", [64, 1], dt.float32)
    b1_d = P("b1", [64, 1], dt.float32)
    w2_d = P("w2", [64, KK, 41])                      # conv2 lhsT [c][tap][m]
    b2_d = P("b2", [41, 1], dt.float32)
    pyb_d = P("pyb", [128, KK, NGK], dt.float32)      # py base (slab coords)
    pxb_d = P("pxb", [128, KK, NGK], dt.float32)
    w2e_d = P("w2e", [C_IN, KK, C_OUT])               # einsum lhsT [c][kk][o]
    dcb_d = P("dcb", [128, 2], dt.float32)
    wdw_d = P("wdw", [C_IN, KK, C_IN])                # dw diag [c][tap][c']
    sdw_d = P("sdw", [C_IN, 1], dt.float32)
    bdw_d = P("bdw", [C_IN, 1], dt.float32)
    wpw_d = P("wpw", [C_IN, C_IN])                    # cb_pw lhsT [c][o]
    bpw_d = P("bpw", [C_IN, 1], dt.float32)
    saw_d = P("saw", [128, 3, 49])                    # sa lhsT [c][kt][dx*7+dy]
    fuw_d = P("fuw", [128, 3, C_OUT])                 # fu lhsT [c][kt][o]
    sf_d = P("sf", [128, 2], dt.float32)
    bf_d = P("bf", [128, 2], dt.float32)
    ones_d = P("ones1", [7, 128])
    mhm_d = P("mhm", [64, 8])                         # valid-row mask top4|bot4
    mcmb_d = P("mcmb", [128, 6])                      # valid-row mask top3|bot3

    out_d = nc.declare_dram_parameter("out", [C_OUT, R_OUT, W], dt.float32,
                                      isOutput=True)
    dbg = {}
    if debug_out:
        for nm, shp, dty in [
                ("d_hm", [64, 1 + R_HM * XC_W + 1], dt.bfloat16),
                ("d_offmask", [41, HW_CMB], dt.bfloat16),
                ("d_wsb", [128, 2 * KK * NGK * 2], dt.bfloat16),
                ("d_idx", [128, KK * NGK], dt.float32),
                ("d_valT", [128, NS], dt.bfloat16),
                ("d_main0", [128, 3 + R_CMB * CMB_W + 3], dt.bfloat16),
                ("d_main1", [128, 3 + R_CMB * CMB_W + 3], dt.bfloat16),
                ("d_contour", [128, 3 + R_CMB * CMB_W + 3], dt.bfloat16),
                ("d_pm49", [49, R_CMB * CMB_W], dt.bfloat16),
                ("d_attn", [1, ACMB], dt.bfloat16),
                ("d_toffs", [128, 41 * NGK], dt.float32)]:
            dbg[nm] = nc.declare_dram_parameter(nm, shp, dty,
                                                isOutput=True)

    with tile.TileContext(nc) as tc:
        with (
            tc.tile_pool(name="const", bufs=1) as const,
            tc.tile_pool(name="work", bufs=1) as work,
            tc.tile_pool(name="gath", bufs=2) as gath,
            tc.tile_pool(name="ps_a", bufs=2, space="PSUM") as ps_a,
            tc.tile_pool(name="ps_b", bufs=2, space="PSUM") as ps_b,
            tc.tile_pool(name="ps_c", bufs=2, space="PSUM") as ps_c,
        ):
            _lp = tc.For_i(0, loop_n) if loop_n > 1 else \
                contextlib.nullcontext()
            with _lp:
                # ---------- load constants ----------
                def load(dram):
                    t = const.tile(list(dram.shape), dram.dtype,
                                   tag=dram.name + "_c", name=dram.name + "_c")
                    nc.sync.dma_start(t[:], dram[:])
                    return t

                xc = load(xc_d)
                w1 = load(w1_d)
                s1 = load(s1_d)
                b1 = load(b1_d)
                w2 = load(w2_d)
                b2 = load(b2_d)
                pyb = load(pyb_d)
                pxb = load(pxb_d)
                w2e = load(w2e_d)
                dcb = load(dcb_d)
                wdw = load(wdw_d)
                sdw = load(sdw_d)
                bdw = load(bdw_d)
                wpw = load(wpw_d)
                bpw = load(bpw_d)
                saw = load(saw_d)
                fuw = load(fuw_d)
                sf = load(sf_d)
                bf = load(bf_d)
                ones = load(ones_d)
                mhm = load(mhm_d)
                mcmb = load(mcmb_d)

                ident = const.tile([128, 128], dt.float32, tag="ident")
                make_identity(nc, ident[:])
                identb = const.tile([128, 128], dt.bfloat16, tag="identb")
                make_identity(nc, identb[:])

                # ---------- conv1: x -> h|hm (64ch, R_HM rows) ----------
                # flat-contiguous rhs over the padded grid (strided matmul rhs
                # hangs HW); pad columns compute garbage that epilogues skip.
                xcf = xc[:]
                hm = work.tile([64, 1 + R_HM * XC_W + 1], dt.bfloat16,
                               tag="hm")
                nc.gpsimd.memset(hm[:], 0.0)

                def conv3x3(src_flat, src_w, lhsT_of, prow0, prow1, epi):
                    # out position p = prow*src_w + col (all cols); rhs flat
                    # slice = src_flat[, p + (ky-1)*src_w + (kx-1)] (+1 pad)
                    chunks = []
                    r = prow0
                    while r < prow1:
                        nr = min(7, prow1 - r)
                        chunks.append((r, nr))
                        r += nr
                    for (r0p, nr) in chunks:
                        n = nr * src_w
                        ps = ps_a.tile([128, 462], dt.float32, tag="pp",
                                       name="ps_cv")
                        for t in range(KK):
                            ky, kx = t // 3, t % 3
                            s0 = 1 + (r0p + ky - 1) * src_w + (kx - 1)
                            nc.tensor.matmul(ps[:lhsT_of(t).shape[-1], :n],
                                             lhsT_of(t),
                                             src_flat[:, s0:s0 + n],
                                             start=(t == 0),
                                             stop=(t == KK - 1))
                        epi(r0p, nr, ps)

                def epi1(r0p, nr, ps):
                    base = 1 + (r0p - 1) * XC_W
                    nc.scalar.activation(
                        hm[:, base:base + nr * XC_W]
                        .rearrange("p (a b) -> p a b", b=XC_W)[:, :, 1:65],
                        ps[0:64, 0:nr * XC_W]
                        .rearrange("p (a b) -> p a b", b=XC_W)[:, :, 1:65],
                        Act.Relu, bias=b1[:], scale=s1[:])

                conv3x3(xcf, XC_W, lambda t: w1[:, t, :], 1, R_X - 1, epi1)

                # zero invalid halo rows (only ever in the top-4/bottom-4)
                hmv = hm[:, 1:1 + R_HM * XC_W]\
                    .rearrange("p (a b) -> p a b", b=XC_W)[:, :, 1:65]
                nc.vector.tensor_tensor(
                    hmv[:, 0:4], hmv[:, 0:4],
                    mhm[:, 0:4].unsqueeze(-1).broadcast_to([64, 4, 64]),
                    Alu.mult)
                nc.vector.tensor_tensor(
                    hmv[:, R_HM - 4:R_HM], hmv[:, R_HM - 4:R_HM],
                    mhm[:, 4:8].unsqueeze(-1).broadcast_to([64, 4, 64]),
                    Alu.mult)

                # ---------- conv2 -> offsets(18)|mask(9), R_CMB rows -------
                offmask = work.tile([41, HW_CMB], dt.bfloat16, tag="offmask")

                def epi2(r0p, nr, ps):
                    j0 = r0p - 1          # offset-row index
                    pv = ps[:, 0:nr * XC_W]\
                        .rearrange("p (a b) -> p a b", b=XC_W)
                    nc.scalar.activation(
                        offmask[0:41, j0 * 64:(j0 + nr) * 64]
                        .rearrange("p (a b) -> p a b", b=64),
                        pv[0:41, :, 1:65],
                        Act.Identity, bias=b2[:], scale=1.0)

                conv3x3(hm[:], XC_W, lambda t: w2[:, t, :], 1, 1 + R_CMB,
                        epi2)

                # ---------- transpose offsets/mask to hw-major ----------
                toffs = work.tile([128, 41, NGK], dt.float32, tag="toffs")
                for g in range(NGK):
                    pst = ps_a.tile([128, 41], dt.bfloat16, tag="ppb")
                    nc.tensor.transpose(pst[:, 0:41],
                                        offmask[:, g * 128:(g + 1) * 128],
                                        identb[0:41, 0:41])
                    nc.vector.tensor_copy(toffs[:, :, g], pst[:, 0:41])

                offy = toffs[:, 0:18:2, :]      # [128, 9, NGK]
                offx = toffs[:, 1:18:2, :]
                msk2 = work.tile([128, KK, NGK], dt.float32, tag="msk2")
                nc.scalar.activation(msk2[:], toffs[:, 32:41, :], Act.Sigmoid)
                maskT = msk2[:]

                # ---------- bilinear fields (hw-major) ----------
                fsh = [128, KK, NGK]

                def ftile(name):
                    return work.tile(fsh, dt.float32, tag=name, name=name)

                py, px = ftile("py"), ftile("px")
                y0f, x0f = ftile("y0f"), ftile("x0f")
                fy, fx = ftile("fy"), ftile("fx")
                ta, tb, tu = ftile("ta"), ftile("tb"), ftile("tu")
                idxf = ftile("idxf")
                y0i = work.tile(fsh, dt.int16, tag="y0i")
                x0i = work.tile(fsh, dt.int16, tag="x0i")
                idxi = work.tile(fsh, dt.int16, tag="idxi")
                # corner weights: [tb][kk][g][x0|x1][dup-pair]
                wsb = work.tile([128, 2, KK, NGK, 2, 2], dt.bfloat16,
                                tag="wsb")

                # floor via fp-add magic: rint(v) = (v + 2^23) - 2^23 in
                # f32 (round-nearest-even, identical on HW and in numpy);
                # floor(py) = rint(py - 0.5) for py >= 0.  The int16 cast
                # then converts an exact integer (rounding-mode-proof).
                MAGIC = float(1 << 23)
                nc.vector.tensor_tensor(py[:], offy, pyb[:], Alu.add)
                nc.vector.tensor_scalar(py[:], py[:], 0.0, float(SLAB_R - 2),
                                        Alu.max, Alu.min)
                nc.vector.tensor_scalar(tu[:], py[:], 0.5, MAGIC,
                                        Alu.subtract, Alu.add)
                nc.vector.tensor_scalar(y0f[:], tu[:], MAGIC, None,
                                        Alu.subtract)
                nc.vector.tensor_copy(y0i[:], y0f[:])
                nc.vector.tensor_tensor(fy[:], py[:], y0f[:], Alu.subtract)

                nc.vector.tensor_tensor(px[:], offx, pxb[:], Alu.add)
                nc.vector.tensor_scalar(px[:], px[:], 0.0, float(SLAB_W - 2),
                                        Alu.max, Alu.min)
                nc.vector.tensor_scalar(tu[:], px[:], 0.5, MAGIC,
                                        Alu.subtract, Alu.add)
                nc.vector.tensor_scalar(x0f[:], tu[:], MAGIC, None,
                                        Alu.subtract)
                nc.vector.tensor_copy(x0i[:], x0f[:])
                nc.vector.tensor_tensor(fx[:], px[:], x0f[:], Alu.subtract)

                nc.vector.scalar_tensor_tensor(idxf[:], y0f[:], float(SLAB_W),
                                               x0f[:], Alu.mult, Alu.add)
                nc.vector.tensor_copy(idxi[:], idxf[:])

                # corner weights (mask folded): w00=m(1-fy)(1-fx) etc.
                # each weight is stored as an adjacent duplicated pair so
                # the mul's weight operand has a packed last dim (stride 1,
                # count 2) -> DVE 2x mode applies despite the broadcast.
                nc.vector.tensor_tensor(tb[:], maskT, fy[:], Alu.mult)
                nc.vector.tensor_tensor(ta[:], maskT, tb[:], Alu.subtract)
                nc.vector.tensor_tensor(tu[:], ta[:], fx[:], Alu.mult)
                for j in range(2):
                    nc.vector.tensor_copy(wsb[:, 0, :, :, 1, j], tu[:])
                nc.vector.tensor_tensor(tu[:], ta[:], tu[:], Alu.subtract)
                for j in range(2):
                    nc.vector.tensor_copy(wsb[:, 0, :, :, 0, j], tu[:])
                nc.vector.tensor_tensor(tu[:], tb[:], fx[:], Alu.mult)
                for j in range(2):
                    nc.vector.tensor_copy(wsb[:, 1, :, :, 1, j], tu[:])
                nc.vector.tensor_tensor(tu[:], tb[:], tu[:], Alu.subtract)
                for j in range(2):
                    nc.vector.tensor_copy(wsb[:, 1, :, :, 0, j], tu[:])

                # ---------- wrap + replicate indices for dma_gather -------
                # idxw_t[16r+p16, kk, g, q] = idx of sample g*128 + 16q +
                # p16; the gather's j-th index lives at partition j%16,
                # free j//16.
                idxw_t = work.tile([128, KK, NGK, 8], dt.int16, tag="idxw_t")
                idxw_b = work.tile([128, KK, NGK, 8], dt.int16, tag="idxw_b")
                for q in range(8):
                    nc.sync.dma_start(idxw_t[0:16, :, :, q],
                                      idxi[16 * q:16 * (q + 1), :, :])
                nc.vector.tensor_scalar(idxw_b[0:16], idxw_t[0:16],
                                        SLAB_W, None, Alu.add)
                for t_ in (idxw_t, idxw_b):
                    nc.sync.dma_start(t_[16:32], t_[0:16])
                    nc.sync.dma_start(t_[32:64], t_[0:32])
                    nc.sync.dma_start(t_[64:128], t_[0:64])

                def cmb_grid(t):      # [128, R_CMB, CMB_W] view of flat tile
                    return t[:, 3:3 + R_CMB * CMB_W]\
                        .rearrange("p (a b) -> p a b", b=CMB_W)

                mm_chunks = [(0, 512), (512, 512), (1024, 512), (1536, 512),
                             (2048, 384)]

                # ---------- contour branch ----------
                hc = work.tile([C_IN, HW_CMB], dt.bfloat16, tag="hc")

                def epi_dw(r0p, nr, ps):
                    j0 = r0p - 2
                    nc.scalar.activation(
                        hc[:, j0 * 64:(j0 + nr) * 64]
                        .rearrange("p (a b) -> p a b", b=64),
                        ps[:, 0:nr * XC_W]
                        .rearrange("p (a b) -> p a b", b=XC_W)[:, :, 1:65],
                        Act.Relu, bias=bdw[:], scale=sdw[:])

                conv3x3(xcf, XC_W, lambda t: wdw[:, t, :], 2, 2 + R_CMB,
                        epi_dw)
                contour = work.tile([C_IN, 3 + R_CMB * CMB_W + 3],
                                    dt.bfloat16, tag="contour")
                nc.gpsimd.memset(contour[:], 0.0)
                for (c0, cn) in mm_chunks:
                    ps = ps_a.tile([C_IN, 512], dt.float32, tag="pp",
                                   name="ps_pw")
                    nc.tensor.matmul(ps[:, :cn], wpw[:], hc[:, c0:c0 + cn],
                                     start=True, stop=True)
                    r0, nr = c0 // 64, cn // 64
                    nc.scalar.activation(
                        cmb_grid(contour)[:, r0:r0 + nr, 3:67],
                        ps[:, :cn].rearrange("p (a b) -> p a b", b=64),
                        Act.Identity, bias=bpw[:], scale=1.0)

                # ---------- per-kk: gather, weight, transpose-accum -------
                xs_flat = AP(tensor=xs_d, offset=0,
                             ap=[[C_IN, N_SLAB], [1, 2 * C_IN]])
                valT = work.tile([128, 1, NS], dt.bfloat16, tag="valT")
                for kk_i in range(KK):
                    # rows 0:19 = top corner pair, 19:38 = bottom.
                    # <=1024 idxs per gather: the 16KB DMA-descriptor
                    # carveout (16B/desc) caps one instruction at ~1024.
                    gtb = gath.tile([128, 2 * NGK, 256], dt.bfloat16,
                                    tag="gtb")
                    for (g0, ng) in [(0, 8), (8, 8), (16, 3)]:
                        ni = ng * 128
                        nc.gpsimd.dma_gather(
                            gtb[:, g0:g0 + ng, :], xs_flat,
                            idxw_t[:, kk_i, g0:g0 + ng, :], ni, ni,
                            elem_size=256, elem_step=C_IN)
                        nc.gpsimd.dma_gather(
                            gtb[:, NGK + g0:NGK + g0 + ng, :], xs_flat,
                            idxw_b[:, kk_i, g0:g0 + ng, :], ni, ni,
                            elem_size=256, elem_step=C_IN)

                    # apply corner weights in place; channel-pair inner
                    # dim keeps all operands packed -> DVE 2x mode
                    for tb_i in range(2):
                        g_v = gtb[:, tb_i * NGK:(tb_i + 1) * NGK, :]\
                            .rearrange("p a (b c d) -> p a b c d", b=2, d=2)
                        w_v = wsb[:, tb_i, kk_i].unsqueeze(3)\
                            .broadcast_to([128, NGK, 2, 64, 2])
                        nc.vector.tensor_tensor(g_v, g_v, w_v, Alu.mult)

                    # per 128-sample group: 4 matmul-"transposes" (rhs =
                    # identity) accumulate the 4 corners into PSUM
                    for gch in range(5):          # 4+4+4+4+3 groups of 128
                        nu = 4 if gch < 4 else 3
                        pst = ps_c.tile([128, 512], dt.float32, tag="tp2",
                                        name="pst2")
                        for u in range(nu):
                            g = gch * 4 + u
                            po = pst[:, u * 128:(u + 1) * 128]
                            for ci, lhs in enumerate((
                                    gtb[:, g, 0:128],
                                    gtb[:, g, 128:256],
                                    gtb[:, NGK + g, 0:128],
                                    gtb[:, NGK + g, 128:256])):
                                nc.tensor.matmul(po, lhs, identb[:],
                                                 start=(ci == 0),
                                                 stop=(ci == 3))
                        nc.scalar.activation(
                            valT[:, 0, kk_i * HW_CMB + gch * 512:
                                 kk_i * HW_CMB + gch * 512 + nu * 128],
                            pst[:, :nu * 128], Act.Copy)

                # ---------- main einsum ----------
                main_sb = []
                for hf in range(2):
                    m_t = work.tile([128, 3 + R_CMB * CMB_W + 3], dt.bfloat16,
                                    tag=f"main{hf}")
                    nc.gpsimd.memset(m_t[:], 0.0)
                    main_sb.append(m_t)

                for (c0, cn) in mm_chunks:
                    for hf in range(2):
                        ps = ps_b.tile([128, 512], dt.float32, tag="mm")
                        for kk_i in range(KK):
                            rhs = valT[:, 0, kk_i * HW_CMB + c0:
                                       kk_i * HW_CMB + c0 + cn]
                            nc.tensor.matmul(
                                ps[:, :cn],
                                w2e[:, kk_i, hf * 128:(hf + 1) * 128],
                                rhs, start=(kk_i == 0), stop=(kk_i == KK - 1))
                        r0, nr = c0 // 64, cn // 64
                        nc.scalar.activation(
                            cmb_grid(main_sb[hf])[:, r0:r0 + nr, 3:67],
                            ps[:, :cn].rearrange("p (a b) -> p a b", b=64),
                            Act.Identity, bias=dcb[:, hf:hf + 1], scale=1.0)

                # ---------- zero invalid rows (only top-3/bottom-3) -------
                cmb = [main_sb[0], main_sb[1], contour]
                for cti in range(3):
                    ctv = cmb_grid(cmb[cti])[:, :, 3:67]
                    nc.vector.tensor_tensor(
                        ctv[:, 0:3], ctv[:, 0:3],
                        mcmb[:, 0:3].unsqueeze(-1)
                        .broadcast_to([128, 3, 64]), Alu.mult)
                    nc.vector.tensor_tensor(
                        ctv[:, R_CMB - 3:R_CMB], ctv[:, R_CMB - 3:R_CMB],
                        mcmb[:, 3:6].unsqueeze(-1)
                        .broadcast_to([128, 3, 64]), Alu.mult)

                # ---------- attention: 7x7 conv -> 1 channel ----------
                # pm49[dx*7+dy, j*70+c'] = sum_c saw[c,kt,dx*7+dy]*cmb over
                # kt blocks; then fold dx (DVE, shifted), fold dy (PE, K=1
                # accumulated matmuls on contiguous windows), sigmoid.
                NPM = R_CMB * CMB_W                     # 2660
                pm49 = work.tile([49, NPM], dt.bfloat16, tag="pm49")
                a_chunks = [(0, 448), (448, 448), (896, 448), (1344, 448),
                            (1792, 448), (2240, 420)]
                for (i0, n) in a_chunks:
                    ps = ps_a.tile([49, 448], dt.float32, tag="pp",
                                   name="ps_at")
                    for kt in range(3):
                        nc.tensor.matmul(ps[:, :n], saw[:, kt, :],
                                         cmb[kt][:, 3 + i0:3 + i0 + n],
                                         start=(kt == 0), stop=(kt == 2))
                    nc.scalar.activation(pm49[:, i0:i0 + n], ps[:, :n],
                                         Act.Copy)

                # dx-fold on PE: 7 accumulated matmuls, lhsT =
                # identity column-block selecting partitions dx*7..dx*7+7,
                # rhs = pm49 shifted by dx in the free dim.
                # pm7[dy, q] = sum_dx pm49[dx*7+dy, q+dx],  q in [0, NP7)
                NP7 = NPM - 6
                pm7 = work.tile([7, NPM], dt.bfloat16, tag="pm7")
                nc.gpsimd.memset(pm7[:, NP7:NPM], 0.0)
                x_chunks = [(0, 448), (448, 448), (896, 448), (1344, 448),
                            (1792, 448), (2240, NP7 - 2240)]
                for (i0, n) in x_chunks:
                    ps = ps_a.tile([7, 448], dt.float32, tag="pp",
                                   name="ps_dx")
                    for dx in range(7):
                        nc.tensor.matmul(
                            ps[:, :n], identb[0:49, 7 * dx:7 * dx + 7],
                            pm49[:, i0 + dx:i0 + dx + n],
                            start=(dx == 0), stop=(dx == 6))
                    nc.scalar.activation(pm7[:, i0:i0 + n], ps[:, :n],
                                         Act.Copy)

                # dy-fold on PE: 7 accumulated matmuls, lhsT = identity
                # column dy (selects partition dy), rhs = pm7 shifted by
                # dy*70; sigmoid.
                # attn[q=r*70+c'] = sig(sum_dy pm7[dy, q + dy*70])
                attn = work.tile([1, ACMB], dt.bfloat16, tag="attn")
                f_chunks = [(0, 448), (448, 448), (896, 448), (1344, 448),
                            (1792, 448)]
                for (i0, n) in f_chunks:
                    ps = ps_a.tile([1, 448], dt.float32, tag="pp",
                                   name="ps_dy")
                    for dy in range(7):
                        nc.tensor.matmul(
                            ps[:, :n], identb[0:7, dy:dy + 1],
                            pm7[0:7, dy * CMB_W + i0:dy * CMB_W + i0 + n],
                            start=(dy == 0), stop=(dy == 6))
                    nc.scalar.activation(attn[:, i0:i0 + n], ps[:, :n],
                                         Act.Sigmoid)

                # replicate attn to 128 partitions via K=1 matmul
                attn_r = work.tile([128, ACMB], dt.bfloat16, tag="attn_r")
                for (i0, n) in f_chunks:
                    ps = ps_a.tile([128, 448], dt.float32, tag="pp",
                                   name="ps_rp")
                    nc.tensor.matmul(ps[:, :n], ones[0:1, :],
                                     attn[:, i0:i0 + n],
                                     start=True, stop=True)
                    nc.scalar.activation(attn_r[:, i0:i0 + n], ps[:, :n],
                                         Act.Copy)

                # ---------- fusion (attn commuted past the 1x1 conv) ------
                for hf in range(2):
                    fvt = work.tile([128, ACMB], dt.bfloat16, tag="fvt")
                    for (i0, n) in f_chunks:
                        ps = ps_b.tile([128, 448], dt.float32, tag="mm")
                        for kt in range(3):
                            rhs = cmb[kt][:, 3 + 3 * CMB_W + i0:
                                          3 + 3 * CMB_W + i0 + n]
                            nc.tensor.matmul(
                                ps[:, :n],
                                fuw[:, kt, hf * 128:(hf + 1) * 128],
                                rhs, start=(kt == 0), stop=(kt == 2))
                        nc.scalar.activation(fvt[:, i0:i0 + n], ps[:, :n],
                                             Act.Copy)
                    # attn multiply: both in 70-col layout, 3-col offset
                    fm = work.tile([128, R_OUT, 64], dt.bfloat16, tag="fm")
                    nc.vector.tensor_tensor(
                        fm[:],
                        fvt[:].rearrange("p (a b) -> p a b",
                                         b=CMB_W)[:, :, 3:67],
                        attn_r[:].rearrange("p (a b) -> p a b",
                                            b=CMB_W)[:, :, 0:64], Alu.mult)
                    outt = work.tile([128, R_OUT * W], dt.float32, tag="outt")
                    nc.scalar.activation(
                        outt[:].rearrange("p (a b) -> p a b", b=64), fm[:],
                        Act.Relu, bias=bf[:, hf:hf + 1],
                        scale=sf[:, hf:hf + 1])
                    nc.sync.dma_start(
                        out_d[hf * 128:(hf + 1) * 128, :, :],
                        outt[:].rearrange("p (a b) -> p a b", b=64))

                if debug_out:
                    def dump(nm, ap):
                        if len(ap.shape) > 2:
                            ap = ap.rearrange("p ... -> p (...)")
                        nc.sync.dma_start(dbg[nm][:], ap)
                    dump("d_hm", hm[:])
                    dump("d_offmask", offmask[:])
                    dump("d_wsb", wsb[:])
                    dump("d_idx", idxf[:])
                    dump("d_valT", valT[:])
                    dump("d_main0", main_sb[0][:])
                    dump("d_main1", main_sb[1][:])
                    dump("d_contour", contour[:])
                    dump("d_pm49", pm49[:])
                    dump("d_attn", attn[:])
                    dump("d_toffs", toffs[:])

    nc.compile()
    return nc


# ---------------- host-side input prep ----------------

def prep_core_inputs(d, core_id):
    b, half = core_id // 2, core_id % 2
    r0 = half * R_OUT

    x = _f32(d["x"][b])                       # [C_IN, H, W]

    xcg = np.zeros((C_IN, R_X, XC_W), np.float32)
    lo, hi = r0 - 5, r0 + R_OUT + 5
    slo, shi = max(lo, 0), min(hi, H)
    xcg[:, slo - lo:shi - lo, 1:65] = x[:, slo:shi, :]
    xc = np.zeros((C_IN, 1 + R_X * XC_W + 1), np.float32)
    xc[:, 1:1 + R_X * XC_W] = xcg.reshape(C_IN, -1)

    xs = np.zeros((N_SLAB + 1, C_IN), np.float32)
    lo2, hi2 = r0 - 8, r0 + R_OUT + 8
    slo2, shi2 = max(lo2, 0), min(hi2, H)
    xsv = xs[:N_SLAB].reshape(SLAB_R, SLAB_W, C_IN)
    xsv[slo2 - lo2:shi2 - lo2, 4:68, :] = \
        x[:, slo2:shi2, :].transpose(1, 2, 0)

    w1 = np.zeros((C_IN, KK, 64), np.float32)
    for t in range(KK):
        ky, kx = t // 3, t % 3
        w1[:, t, 0:32] = d["oc1_w"][:, :, ky, kx].T
        w1[:, t, 32:64] = d["mc1_w"][:, :, ky, kx].T
    sc_o = d["obn_g"] / np.sqrt(d["obn_v"] + EPS)
    bi_o = (d["oc1_b"] - d["obn_m"]) * sc_o + d["obn_b"]
    sc_m = d["mbn_g"] / np.sqrt(d["mbn_v"] + EPS)
    bi_m = (d["mc1_b"] - d["mbn_m"]) * sc_m + d["mbn_b"]
    s1 = np.concatenate([sc_o, sc_m])[:, None]
    b1 = np.concatenate([bi_o, bi_m])[:, None]

    w2 = np.zeros((64, KK, 41), np.float32)
    for t in range(KK):
        ky, kx = t // 3, t % 3
        w2[0:32, t, 0:18] = d["oc2_w"][:, :, ky, kx].T
        w2[32:64, t, 32:41] = d["mc2_w"][:, :, ky, kx].T
    b2 = np.zeros((41, 1), np.float32)
    b2[0:18, 0] = d["oc2_b"]
    b2[32:41, 0] = d["mc2_b"]

    kk = np.arange(KK)
    hw = np.arange(HW_CMB)
    r_i, w_i = hw // 64, hw % 64
    pyb = (r_i[None, :] + 4 + (kk // 3)[:, None]).astype(np.float32)
    pxb = (w_i[None, :] + 3 + (kk % 3)[:, None]).astype(np.float32)
    pyb = pyb.reshape(KK, NGK, 128).transpose(2, 0, 1)
    pxb = pxb.reshape(KK, NGK, 128).transpose(2, 0, 1)

    w2e = d["dc_w"].reshape(C_OUT, C_IN, KK).transpose(1, 2, 0)

    wdw = np.zeros((C_IN, KK, C_IN), np.float32)
    for t in range(KK):
        ky, kx = t // 3, t % 3
        np.fill_diagonal(wdw[:, t, :], d["cb_dw_w"][:, 0, ky, kx])
    sc_c = d["cbn_g"] / np.sqrt(d["cbn_v"] + EPS)
    bi_c = (d["cb_dw_b"] - d["cbn_m"]) * sc_c + d["cbn_b"]

    wpw = d["cb_pw_w"][:, :, 0, 0].T
    # saw[c, kt, dx*7+dy] = sa_w[0, kt*128+c, dy, dx]
    saw = d["sa_w"][0].reshape(3, 128, 7, 7).transpose(1, 0, 3, 2)\
        .reshape(128, 3, 49)
    fuw = d["fu_w"][:, :, 0, 0].T.reshape(3, 128, C_OUT).transpose(1, 0, 2)
    sc_f = d["fbn_g"] / np.sqrt(d["fbn_v"] + EPS)
    bi_f = (d["fu_b"] - d["fbn_m"]) * sc_f + d["fbn_b"]

    rows_hm = np.arange(r0 - 4, r0 + R_OUT + 4)
    vhm = ((rows_hm >= 0) & (rows_hm < H)).astype(np.float32)
    mhm = np.broadcast_to(np.concatenate([vhm[0:4], vhm[-4:]]),
                          (64, 8)).copy()
    rows_cmb = np.arange(r0 - 3, r0 + R_OUT + 3)
    vcmb = ((rows_cmb >= 0) & (rows_cmb < H)).astype(np.float32)
    mcmb = np.broadcast_to(np.concatenate([vcmb[0:3], vcmb[-3:]]),
                           (128, 6)).copy()

    return {
        "xc": _bf16(xc), "xs": _bf16(xs),
        "w1": _bf16(w1), "s1": _f32(s1), "b1": _f32(b1),
        "w2": _bf16(w2), "b2": _f32(b2),
        "pyb": _f32(pyb), "pxb": _f32(pxb),
        "w2e": _bf16(w2e), "dcb": _f32(d["dc_b"].reshape(2, 128).T),
        "wdw": _bf16(wdw), "sdw": _f32(sc_c[:, None]),
        "bdw": _f32(bi_c[:, None]),
        "wpw": _bf16(wpw), "bpw": _f32(d["cb_pw_b"][:, None]),
        "saw": _bf16(saw), "fuw": _bf16(fuw),
        "sf": _f32(sc_f.reshape(2, 128).T), "bf": _f32(bi_f.reshape(2, 128).T),
        "ones1": _bf16(np.ones((7, 128), np.float32)),
        "mhm": _bf16(mhm), "mcmb": _bf16(mcmb),
    }


_NC_CACHE = {}


def get_nc():
    if "nc" not in _NC_CACHE:
        _NC_CACHE["nc"] = build_bass()
    return _NC_CACHE["nc"]


def kernel(**inputs):
    from concourse.bass_utils import run_bass_kernel_spmd

    nc = get_nc()
    d = {k: np.asarray(v) for k, v in inputs.items()}
    in_maps = [prep_core_inputs(d, c) for c in range(8)]
    res = run_bass_kernel_spmd(nc, in_maps, core_ids=list(range(8)))

    out = np.zeros((B, C_OUT, H, W), np.float32)
    for c in range(8):
        b, half = c // 2, c % 2
        out[b, :, half * R_OUT:(half + 1) * R_OUT, :] = res.results[c]["out"]
    return out
